# revision 44
# baseline (speedup 1.0000x reference)
"""Sliding-window causal attention (B=1, S=4096, E=1024, H=16, D=64,
window=(256,0)) on 8 TRN2 NeuronCores.

Sharding: pure sequence-parallel. Core c computes queries [512c, 512c+512)
and needs keys [512c-256, 512c+512) -- a 256-row halo. No collectives.

v3 restructure (trace-driven, from the v2 baseline at 160us):
 - The v2 trace showed PE at ~28% occupancy for the first 30us (input
   stream pacing) and an 11.5us tail.  v3 reorders the whole kernel into
   two phases with a consumption-ordered DMA stream:
   Phase A: Q/K projections + RoPE + scores + exp + mask for ALL heads.
     Wq/Wk are shipped as per-output-block panels so the first projection
     needs only x + one 256KB panel; the PE goes dense at ~3us.
   Phase B: V projection + PV + out-projection.  wv/wo park late in the
     stream (first needed at ~50us).  All 16 masked-probability tiles
     (6MB) stay resident in SBUF between phases.
 - bv is folded into the output bias on the host (bo' = Wo^T bv + bo,
   exact since softmax rows sum to 1): no bv DMA, no V bias adds.
 - Out-projection: 5 PSUM banks hold partial accumulations (et=0..5/6)
   while the last head pairs finish; the tail is one rank-update + one
   evacuation per output tile, split across ACT and DVE, DMAs staggered.
 - wo reuses the wq panel buffers (tag rotation => automatic WAR dep,
   the DMA starts only once the last Q projection has read the panel).
"""

import os
import sys

sys.path.insert(0, "/opt/trn_rl_repo")

import math

import numpy as np
import ml_dtypes

import concourse.bass as bass
from concourse import bacc
import concourse.mybir as mybir
import concourse.tile as tile
from concourse.bass_utils import run_bass_kernel_spmd


def _ensure_ntff_hook():
    """Register the axon NTFF profile hook so trace=True works."""
    import types
    try:
        import antenv
    except ImportError:
        return
    if "antenv.axon_hooks" not in sys.modules:
        mod = types.ModuleType("antenv.axon_hooks")
        mod._hook = None
        def _set(h, _m=mod):
            _m._hook = h
        def _get(_m=mod):
            return _m._hook
        mod.set_axon_ntff_profile_hook = _set
        mod.get_axon_ntff_profile_hook = _get
        sys.modules["antenv.axon_hooks"] = mod
        antenv.axon_hooks = mod
    hooks = sys.modules["antenv.axon_hooks"]
    if hooks.get_axon_ntff_profile_hook() is None:
        try:
            from trn_agent_boot.trn_boot import _ntff_profile_via_ctypes
            hooks.set_axon_ntff_profile_hook(
                _ntff_profile_via_ctypes("/opt/axon/libaxon_pjrt.so"))
        except Exception:
            pass

BF16 = mybir.dt.bfloat16
F32 = mybir.dt.float32

NCORES = 8
S = 4096
E = 1024
H = 16
D = 64
SL = S // NCORES        # 512 local queries per core
HALO = 256
SK = SL + HALO          # 768 local keys (padded frame)
NQT = SL // 128         # 4 query tiles
NKT = SK // 128         # 6 key tiles
NET = E // 128          # 8 embed tiles
SCALE = 1.0 / math.sqrt(D)
NWARM = 16              # PE warm-up matmuls: a serial chain opens the
                        # HAM clock gate (~1.2GHz until it opens) early

# Bank-packed score layout for one head: each k-tile's whole contiguous
# query range is ONE matmul; the 6 ranges fill 3 PSUM banks of 512 f32
# exactly.  Each entry is one matmul: (bank, bank_col, kt, qlo, qhi).
SCORE_BLOCKS = [
    (0, 0,   2, 0,   384),
    (0, 384, 0, 0,   128),
    (1, 0,   3, 128, 512),
    (1, 384, 5, 384, 512),
    (2, 0,   1, 0,   256),
    (2, 256, 4, 256, 512),
]
# (kt, qtile) -> column offset of that 128-col block in the packed
# [128, 1536] probability tile.
PV_OFF = {}
for _b, _c, _kt, _qlo, _qhi in SCORE_BLOCKS:
    for _i in range((_qhi - _qlo) // 128):
        PV_OFF[(_kt, _qlo // 128 + _i)] = _b * 512 + _c + _i * 128


def _build_graph():
    nc = bacc.Bacc("TRN2", target_bir_lowering=False, debug=False, num_devices=NCORES)

    # ---- DRAM parameters (per-core shards staged by kernel()) ----
    # x packed as row tiles side by side: col kt*SK+s = x[kt*128+p, s]
    xall = nc.declare_dram_parameter("xall", [128, NET * SK], BF16, isOutput=False)
    # wq/wk packed as per-et panels: col et*E+kt*128+j = W[kt*128+p, et*128+j]
    wqp = nc.declare_dram_parameter("wqp", [128, NET * E], BF16, isOutput=False)
    wkp = nc.declare_dram_parameter("wkp", [128, NET * E], BF16, isOutput=False)
    # wv/wo packed as row tiles: col kt*E+c = W[kt*128+p, c]
    wvp = nc.declare_dram_parameter("wvp", [128, NET * E], BF16, isOutput=False)
    wop = nc.declare_dram_parameter("wop", [128, NET * E], BF16, isOutput=False)
    rt = nc.declare_dram_parameter("rt", [128, 128], BF16, isOutput=False)
    # bq in cols 0..7, bo' = Wo^T bv + bo in cols 8..15
    bqo = nc.declare_dram_parameter("bqo", [128, 2 * NET], F32, isOutput=False)
    csall = nc.declare_dram_parameter("csall", [128, 2 * SK], BF16, isOutput=False)
    maskpat = nc.declare_dram_parameter("maskpat", [128, 1536], BF16, isOutput=False)
    out_ext = nc.declare_dram_parameter("out", [E, SL], BF16, isOutput=True)

    with tile.TileContext(nc) as tc:
        with (
            tc.tile_pool(name="wpool", bufs=1) as wpool,
            tc.tile_pool(name="xpool", bufs=1) as xpool,
            tc.tile_pool(name="qk", bufs=3) as qkpool,
            tc.tile_pool(name="vpool", bufs=1) as vpool,
            tc.tile_pool(name="cs", bufs=1) as cspool,
            tc.tile_pool(name="small", bufs=1) as small,
            tc.tile_pool(name="rope", bufs=2) as ropepool,
            tc.tile_pool(name="pe", bufs=2) as pepool,
            tc.tile_pool(name="pm", bufs=16) as pmpool,
            tc.tile_pool(name="att", bufs=3) as attpool,
            tc.tile_pool(name="ctx", bufs=1) as ctxpool,
            tc.tile_pool(name="outp", bufs=8) as outpool,
            tc.tile_pool(name="mm", bufs=3, space="PSUM") as mmps,
            tc.tile_pool(name="sc", bufs=3, space="PSUM") as scps,
            tc.tile_pool(name="op", bufs=2, space="PSUM") as opps,
        ):
            # ---------- warm-up + DMA stream (consumption order) ----------
            warm_sb = small.tile([128, 512], BF16, tag="warm")
            nc.vector.memset(warm_sb[:], 0.0)
            ones_sb = small.tile([1, 64], BF16, tag="ones")
            nc.vector.memset(ones_sb[:], 1.0)

            warm_ctr = [0]

            def warm_mm():
                i = warm_ctr[0]
                warm_ctr[0] += 1
                wp = mmps.tile([128, 512], F32, tag="mm", name=f"warm{i}")
                nc.tensor.matmul(wp[:], warm_sb[:, 0:128], warm_sb[:],
                                 start=True, stop=True)

            bqo_sb = small.tile([128, 2 * NET], F32, tag="bqo")
            nc.sync.dma_start(bqo_sb[:], bqo[:])

            # x in 4 groups (kt 0, 1-2, 3-5, 6-7): first is small so the
            # first projection matmul can start as early as possible
            XG = [(0, 1), (1, 3), (3, 6), (6, 8)]
            xg_sb = []

            def x_mv(kt):
                for gi, (lo, hi) in enumerate(XG):
                    if lo <= kt < hi:
                        return xg_sb[gi], (kt - lo) * SK
                raise AssertionError

            def dma_xg(gi):
                lo, hi = XG[gi]
                t = xpool.tile([128, (hi - lo) * SK], BF16, tag=f"xg{gi}",
                               name=f"xg{gi}")
                nc.sync.dma_start(t[:], xall[:, lo * SK : hi * SK])
                xg_sb.append(t)

            wq_sb = [None] * NET
            wk_sb = [None] * NET

            def dma_panel(which, et):
                t = wpool.tile([128, E], BF16, tag=f"{'wq' if which == 'q' else 'wk'}{et}",
                               name=f"{which}p{et}")
                src = wqp if which == "q" else wkp
                nc.sync.dma_start(t[:], src[:, et * E : (et + 1) * E])
                (wq_sb if which == "q" else wk_sb)[et] = t

            # stream: bqo, xg0, wq0, xg1, wq1, xg2, wq2, wk0, cs, rt, wq3,
            # wk1, mask, wq4, wk2, wq5, wk3, wv0, wv1, wq6, wk4, wq7, wk5,
            # wk6, wk7, wo0..7 (wo reuses wq slots -> WAR-deferred)
            dma_xg(0)
            dma_panel("q", 0)
            dma_xg(1)
            dma_panel("q", 1)
            dma_xg(2)
            dma_panel("q", 2)
            dma_xg(3)
            dma_panel("k", 0)
            cs_sb = cspool.tile([128, 2 * SK], BF16, tag="cs")
            nc.sync.dma_start(cs_sb[:], csall[:])
            cos_sb = cs_sb[:, 0:SK]
            sin_sb = cs_sb[:, SK : 2 * SK]
            rt_sb = small.tile([128, 128], BF16, tag="rt")
            nc.sync.dma_start(rt_sb[:], rt[:])
            dma_panel("q", 3)
            dma_panel("k", 1)
            mask_sb = cspool.tile([128, 1536], BF16, tag="mask")
            nc.sync.dma_start(mask_sb[:], maskpat[:])
            dma_panel("q", 4)
            dma_panel("k", 2)
            dma_panel("q", 5)
            dma_panel("k", 3)
            wv_sb = []
            for g in range(2):
                t = vpool.tile([128, 4 * E], BF16, tag=f"wv{g}", name=f"wv{g}")
                nc.sync.dma_start(t[:], wvp[:, g * 4 * E : (g + 1) * 4 * E])
                wv_sb.append(t)
            dma_panel("q", 6)
            dma_panel("k", 4)
            dma_panel("q", 7)
            dma_panel("k", 5)
            dma_panel("k", 6)
            dma_panel("k", 7)
            # wo row tiles reuse the wq panel buffers (same tag): the DMA
            # waits until the last Q-projection matmul has read the panel.
            wo_sb = []
            for et in range(NET):
                t = wpool.tile([128, E], BF16, tag=f"wq{et}", name=f"wo{et}")
                nc.sync.dma_start(t[:], wop[:, et * E : (et + 1) * E])
                wo_sb.append(t)

            def wv_mv(kt, half):
                return wv_sb[kt // 4][:, (kt % 4) * E + half * 512 :
                                      (kt % 4) * E + (half + 1) * 512]

            # ---------- phase A: Q/K proj + rope + scores + exp + mask ----
            q_rope = [None] * NET   # rotating pool (bufs=3)
            k_rope = [None] * NET
            pm_tiles = {}

            def proj_q(et, trickle=()):
                qp = mmps.tile([128, 512], F32, tag="mm", name=f"qp{et}")
                for kt in range(NET):
                    xg, base = x_mv(kt)
                    nc.tensor.matmul(
                        qp[:],
                        wq_sb[et][:, kt * 128 : (kt + 1) * 128],
                        xg[:, base + HALO : base + SK],
                        start=(kt == 0),
                        stop=(kt == NET - 1),
                    )
                    if kt in trickle:
                        warm_mm()
                q_lin = ropepool.tile([128, SL], BF16, tag="qlin", bufs=3)
                nc.scalar.activation(
                    q_lin[:], qp[:],
                    mybir.ActivationFunctionType.Identity,
                    bias=bqo_sb[:, et : et + 1], scale=1.0,
                )
                return q_lin

            q_lin_t = [None] * NET

            def proj_k_rope(et):
                # k: all SK rows, no bias
                kp = mmps.tile([128, 512], F32, tag="mm", name=f"kp{et}")
                kp2 = mmps.tile([128, 512], F32, tag="mm", name=f"kp2{et}")
                for kt in range(NET):
                    xg, base = x_mv(kt)
                    nc.tensor.matmul(
                        kp[:],
                        wk_sb[et][:, kt * 128 : (kt + 1) * 128],
                        xg[:, base : base + 512],
                        start=(kt == 0), stop=(kt == NET - 1),
                    )
                for kt in range(NET):
                    xg, base = x_mv(kt)
                    nc.tensor.matmul(
                        kp2[:, 0 : SK - 512],
                        wk_sb[et][:, kt * 128 : (kt + 1) * 128],
                        xg[:, base + 512 : base + SK],
                        start=(kt == 0), stop=(kt == NET - 1),
                    )
                k_lin = ropepool.tile([128, SK], BF16, tag="klin")
                nc.scalar.copy(k_lin[:, 0:512], kp[:, 0:512])
                nc.scalar.copy(k_lin[:, 512:SK], kp2[:, 0 : SK - 512])
                q_lin = q_lin_t[et]
                # rotate_half via matmul with the static rotation matrix
                rotp = mmps.tile([128, 512], F32, tag="mm", name="rotp")
                nc.tensor.matmul(rotp[:, 0:SL], rt_sb[:], q_lin[:],
                                 start=True, stop=True)
                rotk = mmps.tile([128, 512], F32, tag="mm", name="rotk")
                nc.tensor.matmul(rotk[:, :], rt_sb[:], k_lin[:, 0:512],
                                 start=True, stop=True)
                rotk2 = mmps.tile([128, 512], F32, tag="mm", name="rotk2")
                nc.tensor.matmul(rotk2[:, 0 : SK - 512], rt_sb[:],
                                 k_lin[:, 512:SK], start=True, stop=True)
                # all-bf16 SBUF temporaries hit the DVE 2x/4x perf modes
                t1 = ropepool.tile([128, SL], BF16, tag="t1", name="t1")
                nc.vector.tensor_mul(t1[:], q_lin[:], cos_sb[:, HALO:SK])
                t2 = ropepool.tile([128, SL], BF16, tag="t2", name="t2")
                nc.vector.tensor_mul(t2[:], rotp[:, 0:SL], sin_sb[:, HALO:SK])
                qf = qkpool.tile([128, SL], BF16, tag="qr", name=f"qf{et}")
                nc.vector.tensor_add(qf[:], t1[:], t2[:])
                q_rope[et] = qf
                t3 = ropepool.tile([128, SK], BF16, tag="t3", name="t3")
                nc.vector.tensor_mul(t3[:], k_lin[:], cos_sb[:])
                t4 = ropepool.tile([128, SK], BF16, tag="t4", name="t4")
                nc.vector.tensor_mul(t4[:, 0:512], rotk[:, :], sin_sb[:, 0:512])
                nc.vector.tensor_mul(t4[:, 512:SK], rotk2[:, 0 : SK - 512],
                                     sin_sb[:, 512:SK])
                kf = qkpool.tile([128, SK], BF16, tag="kr", name=f"kf{et}")
                nc.vector.tensor_add(kf[:], t3[:], t4[:])
                k_rope[et] = kf

            def emit_scores(hp, sub):
                # one head: 8 matmuls bank-pack the 12 band blocks into 3
                # PSUM banks; 3 big exps; one batched mask multiply.
                et = hp
                banks = [scps.tile([128, 512], F32, tag="sc",
                                   name=f"sc{hp}_{sub}_{b}") for b in range(3)]
                for (b, c, kt, qlo, qhi) in SCORE_BLOCKS:
                    nc.tensor.matmul(
                        banks[b][:, c : c + (qhi - qlo)],
                        k_rope[et][sub : sub + 64, kt * 128 : (kt + 1) * 128],
                        q_rope[et][sub : sub + 64, qlo:qhi],
                        start=True, stop=True,
                    )
                pe = pepool.tile([128, 1536], BF16, tag="pe",
                                 name=f"pe{hp}_{sub}")
                for b in range(3):
                    nc.scalar.activation(
                        pe[:, b * 512 : (b + 1) * 512], banks[b][:],
                        mybir.ActivationFunctionType.Exp,
                        bias=0.0, scale=SCALE,
                    )
                pm = pmpool.tile([128, 1536], BF16, tag="pm",
                                 name=f"pm{hp}_{sub}")
                nc.vector.tensor_mul(pm[:], pe[:], mask_sb[:])
                pm_tiles[(hp, sub)] = pm

            for _ in range(NWARM):
                warm_mm()
            q_lin_t[0] = proj_q(0, trickle=(1, 3, 5))
            q_lin_t[1] = proj_q(1, trickle=(2, 5))
            for et in range(NET):
                if et + 2 < NET:
                    q_lin_t[et + 2] = proj_q(et + 2,
                                             trickle=(3,) if et <= 1 else ())
                proj_k_rope(et)
                if et >= 1:
                    emit_scores(et - 1, 0)
                    emit_scores(et - 1, 64)
            emit_scores(NET - 1, 0)
            emit_scores(NET - 1, 64)

            # ---------- phase B: V proj + PV + out projection ----------
            # Per head 128 cols: ones at col 0 (so PV's sum(P) row lands at
            # PSUM partition 0, where the custom reciprocal can read it
            # directly) and the 64 value dims at cols 64..127 (so the
            # context evacuates via an ACT partition-shift copy 64->0).
            # Cols 1..63 stay garbage -- their PSUM rows are never read.
            v_sb = []
            for st in range(NKT):
                vt = vpool.tile([128, 16 * 128], BF16, tag=f"v{st}",
                                name=f"v{st}")
                nc.gpsimd.memset(
                    vt[:].rearrange("p (h c) -> p h c", c=128)[:, :, 0:1], 1.0
                )
                v_sb.append(vt)

            def emit_v(st):
                vt = v_sb[st]
                for half in range(2):
                    vp = mmps.tile([128, 512], F32, tag="mm", name=f"vp{st}")
                    for kt in range(NET):
                        xg, base = x_mv(kt)
                        nc.tensor.matmul(
                            vp[:],
                            xg[:, base + st * 128 : base + (st + 1) * 128],
                            wv_mv(kt, half),
                            start=(kt == 0),
                            stop=(kt == NET - 1),
                        )
                    dst = vt[:, half * 8 * 128 : (half * 8 + 8) * 128].rearrange(
                        "p (h c) -> p h c", c=128
                    )[:, :, 64:128]
                    nc.scalar.copy(dst, vp[:].rearrange("p (h c) -> p h c", c=64))

            ctx_sb = [ctxpool.tile([128, SL], BF16, tag=f"ctx{et}",
                                   name=f"ctx{et}")
                      for et in range(NET)]

            pend_lbc = []

            def pv_alloc(hp, sub):
                return scps.tile([128, 512], F32, tag="sc",
                                 name=f"cx{hp}_{sub}")

            def pv_mm_range(cxp, hp, sub, kts):
                pm = pm_tiles[(hp, sub)]
                h = 2 * hp + sub // 64
                for kt in kts:
                    lo = max(0, kt - 2) * 128
                    hi = min(kt + 1, NQT) * 128
                    off = PV_OFF[(kt, lo // 128)]
                    nc.tensor.matmul(
                        cxp[:, lo:hi],
                        v_sb[kt][:, h * 128 : (h + 1) * 128],
                        pm[:, off : off + (hi - lo)],
                        start=(kt == 0), stop=(kt == NKT - 1),
                    )

            def pv_finish(cxp, hp, sub):
                pm_tiles.pop((hp, sub))
                # sum(P) sits at PSUM partition 0: the custom reciprocal
                # reads it in place (no single-partition DVE copy)
                linv = attpool.tile([1, SL], F32, tag="linv")
                nc.vector.reciprocal_approx_fast(linv[:], cxp[0:1, :])
                # cast on ACT (keeps the in-order DVE queue short)
                linv_bf = attpool.tile([1, SL], BF16, tag="linvb")
                nc.scalar.copy(linv_bf[:], linv[:])
                # context rows evacuate 64->0 via ACT partition-shift copy
                craw = attpool.tile([64, SL], F32, tag="craw")
                nc.scalar.copy(craw[:], cxp[64:128, :])
                pend_lbc.append((hp, sub, linv_bf, craw))

            def flush_lbc():
                """Broadcast 1/l on the PE, finish ctx = craw * (1/l).  Only
                called with >=1 unit of PE work emitted since the PV."""
                hp, sub, linv_bf, craw = pend_lbc.pop(0)
                lbc_ps = scps.tile([128, 512], F32, tag="sc",
                                   name=f"lbc{hp}_{sub}")
                nc.tensor.matmul(lbc_ps[0:64, :], ones_sb[:],
                                 linv_bf[:], start=True, stop=True)
                nc.vector.tensor_mul(ctx_sb[hp][sub : sub + 64, :],
                                     craw[:], lbc_ps[0:64, :])

            def emit_pv_mm(hp, sub):
                """PV matmuls + start of the normalize chain (recip)."""
                cxp = pv_alloc(hp, sub)
                pv_mm_range(cxp, hp, sub, range(NKT))
                pv_finish(cxp, hp, sub)

            def flush_lbc():
                """Broadcast 1/l on the PE, finish ctx = craw * (1/l).  Only
                called with >=1 unit of PE work emitted since the PV."""
                hp, sub, linv_bf, craw = pend_lbc.pop(0)
                lbc_ps = scps.tile([128, 512], F32, tag="sc",
                                   name=f"lbc{hp}_{sub}")
                nc.tensor.matmul(lbc_ps[0:64, :], ones_sb[:],
                                 linv_bf[:], start=True, stop=True)
                nc.vector.tensor_mul(ctx_sb[hp][sub : sub + 64, :],
                                     craw[:], lbc_ps[0:64, :])

            # V projections with the first PV pair's matmuls threaded in:
            # each PV matmul reads v_sb[kt] evacuated two V units earlier,
            # so the junction has no exposed evac wait.
            emit_v(0)
            emit_v(1)
            cx00 = pv_alloc(0, 0)
            emit_v(2)
            pv_mm_range(cx00, 0, 0, [0])
            emit_v(3)
            pv_mm_range(cx00, 0, 0, [1])
            cx064 = pv_alloc(0, 64)
            pv_mm_range(cx064, 0, 64, [0])
            emit_v(4)
            pv_mm_range(cx00, 0, 0, [2])
            pv_mm_range(cx064, 0, 64, [1])
            emit_v(5)
            pv_mm_range(cx00, 0, 0, [3, 4, 5])
            pv_finish(cx00, 0, 0)
            pv_mm_range(cx064, 0, 64, [2, 3, 4, 5])
            pv_finish(cx064, 0, 64)

            # Out-projection partials, spread through the PV phase as lbc
            # cover: 5 held PSUM banks accumulate et chunks as ctx tiles
            # complete (et 0-2 after ctx[2], et 3-5 after ctx[5]).
            op_hold = {}

            def op_chunk(eo, ets, pool=None):
                op = op_hold.get(eo)
                if op is None:
                    op = pool.tile([128, 512], F32,
                                   tag=("op" if pool is opps else "mm"),
                                   name=f"op{eo}")
                    op_hold[eo] = op
                for et in ets:
                    nc.tensor.matmul(
                        op[:],
                        wo_sb[et][:, eo * 128 : (eo + 1) * 128],
                        ctx_sb[et][:],
                        start=(et == 0), stop=False,
                    )

            # Pair k's broadcast flushes after pair k+1's first PV unit
            # (~1.5 units of cover for the recip->cast chain); out partial
            # chunks interleave as additional cover once ctx tiles land.
            OPPOOL = {0: opps, 1: opps, 2: mmps, 3: mmps, 4: mmps}

            def opA(eo):
                op_chunk(eo, range(0, 3), pool=OPPOOL[eo])

            def opB(eo):
                op_chunk(eo, range(3, 6), pool=OPPOOL[eo])

            # lbc lags two units: the recip->cast->broadcast chain gets
            # ~2us of PV/partial matmul cover before the PE needs it
            cover = iter(
                [None] * 6   # until ctx[2] is complete (flush of (2,64))
                + [("A", 0), ("A", 1), ("A", 2), ("A", 3), ("A", 4), None,
                   ("B", 0), ("B", 1), ("B", 2), ("B", 3), ("B", 4)]
            )

            def emit_cover():
                c = next(cover, None)
                if c is None:
                    return
                stage, eo = c
                (opA if stage == "A" else opB)(eo)

            for hp in range(1, NET):
                for sub in (0, 64):
                    emit_pv_mm(hp, sub)
                    if len(pend_lbc) > 2:
                        emit_cover()
                        flush_lbc()
            emit_cover()
            flush_lbc()
            emit_cover()
            flush_lbc()
            for _ in range(10):
                emit_cover()

            def finish_out(eo, op):
                o_sb = outpool.tile([128, SL], BF16, tag="o")
                if eo % 2 == 0:
                    nc.scalar.activation(
                        o_sb[:], op[:], mybir.ActivationFunctionType.Identity,
                        bias=bqo_sb[:, NET + eo : NET + eo + 1], scale=1.0,
                    )
                else:
                    nc.vector.tensor_scalar_add(
                        o_sb[:], op[:], bqo_sb[:, NET + eo : NET + eo + 1],
                    )
                nc.sync.dma_start(out_ext[eo * 128 : (eo + 1) * 128, :], o_sb[:])

            def tail_warm():
                wp = scps.tile([128, 512], F32, tag="sc",
                               name=f"twarm{warm_ctr[0]}")
                warm_ctr[0] += 1
                nc.tensor.matmul(wp[:], warm_sb[:, 0:128], warm_sb[:],
                                 start=True, stop=True)

            # rank updates et=6,7 for the held banks, then evacuate; tail
            # warm matmuls keep the HAM clock open while ACT/DVE/DMA drain
            for eo in range(5):
                op = op_hold[eo]
                for et in (6, 7):
                    nc.tensor.matmul(
                        op[:],
                        wo_sb[et][:, eo * 128 : (eo + 1) * 128],
                        ctx_sb[et][:],
                        start=False, stop=(et == 7),
                    )
                finish_out(eo, op)
                tail_warm()
            # eo=5..7: full chains through freed banks
            for eo, pool, tg in ((5, mmps, "mm"), (6, opps, "op"), (7, opps, "op")):
                op = pool.tile([128, 512], F32, tag=tg, name=f"opf{eo}")
                for et in range(NET):
                    nc.tensor.matmul(
                        op[:],
                        wo_sb[et][:, eo * 128 : (eo + 1) * 128],
                        ctx_sb[et][:],
                        start=(et == 0), stop=(et == NET - 1),
                    )
                finish_out(eo, op)
                tail_warm()
            tail_warm()
            tail_warm()

    nc.compile()
    return nc


_NC_CACHE = None
LAST_RESULT = None


def _get_graph():
    global _NC_CACHE
    if _NC_CACHE is None:
        _NC_CACHE = _build_graph()
    return _NC_CACHE


def _rot_matrix():
    # rot(q)[d] = -q[d+32] (d<32) ; q[d-32] (d>=32), per 64-block; 2 blocks.
    r64 = np.zeros((64, 64), dtype=np.float32)
    for d in range(32):
        r64[d, d + 32] = -1.0
        r64[d + 32, d] = 1.0
    r = np.zeros((128, 128), dtype=np.float32)
    r[0:64, 0:64] = r64
    r[64:128, 64:128] = r64
    return r


def _maskpat(core):
    """Packed [128, 1536] multiplicative window mask for one core.

    Column b*512 + c + i*128 + u corresponds to key row ki of k-tile kt
    against query column (qlo//128 + i)*128 + u; valid iff the key is in
    the causal 256-window and (core 0) not a zero-padded halo row.
    """
    pat = np.zeros((128, 1536), dtype=np.float32)
    ki = np.arange(128)[:, None]
    u = np.arange(128)[None, :]
    for (b, c, kt, qlo, qhi) in SCORE_BLOCKS:
        for i in range((qhi - qlo) // 128):
            qj = qlo + i * 128 + u
            k_pad = kt * 128 + ki
            valid = (qj <= k_pad) & (k_pad <= qj + HALO)
            if core == 0:
                valid = valid & (k_pad >= HALO)
            pat[:, b * 512 + c + i * 128 : b * 512 + c + (i + 1) * 128] = valid
    return pat.astype(ml_dtypes.bfloat16)


def kernel(x, mask, cos, sin, Wq, bq, Wk, Wv, bv, Wo, bo):
    x = np.asarray(x, dtype=np.float32)
    cos = np.asarray(cos, dtype=np.float32)
    sin = np.asarray(sin, dtype=np.float32)
    B = x.shape[0]
    assert (B, S, E) == x.shape

    bf = lambda a: np.ascontiguousarray(a).astype(ml_dtypes.bfloat16)
    Wq = np.asarray(Wq, np.float32)
    Wk = np.asarray(Wk, np.float32)
    Wv = np.asarray(Wv, np.float32)
    Wo = np.asarray(Wo, np.float32)
    # per-et panels: [p, et, kt, j]
    wqp_b = bf(Wq.reshape(NET, 128, NET, 128).transpose(1, 2, 0, 3)
               .reshape(128, NET * E))
    wkp_b = bf(Wk.reshape(NET, 128, NET, 128).transpose(1, 2, 0, 3)
               .reshape(128, NET * E))
    # row-tile packs: [p, kt, c]
    wvp_b = bf(Wv.reshape(NET, 128, E).transpose(1, 0, 2).reshape(128, NET * E))
    wop_b = bf(Wo.reshape(NET, 128, E).transpose(1, 0, 2).reshape(128, NET * E))
    rt_b = bf(_rot_matrix().T)
    # fold the V bias through the output projection: ctx rows sum to 1
    bo_f = np.asarray(bo, np.float32) + Wo.T @ np.asarray(bv, np.float32)
    bqo_t = np.concatenate(
        [np.asarray(bq, np.float32).reshape(NET, 128).T,
         bo_f.reshape(NET, 128).T], axis=1)
    bqo_t = np.ascontiguousarray(bqo_t)

    in_maps = []
    for c in range(NCORES):
        lo = c * SL - HALO
        xp = np.zeros((SK, E), dtype=np.float32)
        cp = np.zeros((SK, D), dtype=np.float32)
        sp = np.zeros((SK, D), dtype=np.float32)
        src_lo = max(lo, 0)
        dst_lo = src_lo - lo
        xp[dst_lo:] = x[0, src_lo : lo + SK]
        cp[dst_lo:] = cos[0, src_lo : lo + SK]
        sp[dst_lo:] = sin[0, src_lo : lo + SK]
        xall_b = bf(xp.T.reshape(NET, 128, SK).transpose(1, 0, 2)
                    .reshape(128, NET * SK))
        cs_b = np.concatenate(
            [np.tile(cp.T, (2, 1)), np.tile(sp.T, (2, 1))], axis=1)
        in_maps.append({
            "xall": xall_b,
            "wqp": wqp_b, "wkp": wkp_b, "wvp": wvp_b, "wop": wop_b,
            "rt": rt_b,
            "bqo": bqo_t,
            "csall": bf(cs_b),
            "maskpat": _maskpat(c),
        })

    nc = _get_graph()
    trace = bool(os.environ.get("BASS_KERNEL_TRACE"))
    if trace:
        _ensure_ntff_hook()
    res = run_bass_kernel_spmd(
        nc, in_maps, core_ids=list(range(NCORES)), trace=trace
    )
    global LAST_RESULT
    LAST_RESULT = res

    out = np.empty((1, S, E), dtype=np.float32)
    for c in range(NCORES):
        out[0, c * SL : (c + 1) * SL, :] = (
            res.results[c]["out"].astype(np.float32).T)
    return out


if __name__ == "__main__":
    import reference
    inputs = reference.setup_inputs()
    inputs = {k: np.asarray(v) for k, v in inputs.items()}
    got = kernel(**inputs)
    exp = np.asarray(reference.reference(**inputs))
    err = np.abs(got - exp).max() / np.abs(exp).max()
    print("rel err:", err)


# revision 47
# speedup vs baseline: 1.0378x; 1.0378x over previous
"""Sliding-window causal attention (B=1, S=4096, E=1024, H=16, D=64,
window=(256,0)) on 8 TRN2 NeuronCores.

Sharding: pure sequence-parallel. Core c computes queries [512c, 512c+512)
and needs keys [512c-256, 512c+512) -- a 256-row halo. No collectives.

v3 restructure (trace-driven, from the v2 baseline at 160us):
 - The v2 trace showed PE at ~28% occupancy for the first 30us (input
   stream pacing) and an 11.5us tail.  v3 reorders the whole kernel into
   two phases with a consumption-ordered DMA stream:
   Phase A: Q/K projections + RoPE + scores + exp + mask for ALL heads.
     Wq/Wk are shipped as per-output-block panels so the first projection
     needs only x + one 256KB panel; the PE goes dense at ~3us.
   Phase B: V projection + PV + out-projection.  wv/wo park late in the
     stream (first needed at ~50us).  All 16 masked-probability tiles
     (6MB) stay resident in SBUF between phases.
 - bv is folded into the output bias on the host (bo' = Wo^T bv + bo,
   exact since softmax rows sum to 1): no bv DMA, no V bias adds.
 - Out-projection: 5 PSUM banks hold partial accumulations (et=0..5/6)
   while the last head pairs finish; the tail is one rank-update + one
   evacuation per output tile, split across ACT and DVE, DMAs staggered.
 - wo reuses the wq panel buffers (tag rotation => automatic WAR dep,
   the DMA starts only once the last Q projection has read the panel).
"""

import os
import sys

sys.path.insert(0, "/opt/trn_rl_repo")

import math

import numpy as np
import ml_dtypes

import concourse.bass as bass
from concourse import bacc
import concourse.mybir as mybir
import concourse.tile as tile
from concourse.bass_utils import run_bass_kernel_spmd


def _ensure_ntff_hook():
    """Register the axon NTFF profile hook so trace=True works."""
    import types
    try:
        import antenv
    except ImportError:
        return
    if "antenv.axon_hooks" not in sys.modules:
        mod = types.ModuleType("antenv.axon_hooks")
        mod._hook = None
        def _set(h, _m=mod):
            _m._hook = h
        def _get(_m=mod):
            return _m._hook
        mod.set_axon_ntff_profile_hook = _set
        mod.get_axon_ntff_profile_hook = _get
        sys.modules["antenv.axon_hooks"] = mod
        antenv.axon_hooks = mod
    hooks = sys.modules["antenv.axon_hooks"]
    if hooks.get_axon_ntff_profile_hook() is None:
        try:
            from trn_agent_boot.trn_boot import _ntff_profile_via_ctypes
            hooks.set_axon_ntff_profile_hook(
                _ntff_profile_via_ctypes("/opt/axon/libaxon_pjrt.so"))
        except Exception:
            pass

BF16 = mybir.dt.bfloat16
F32 = mybir.dt.float32

NCORES = 8
S = 4096
E = 1024
H = 16
D = 64
SL = S // NCORES        # 512 local queries per core
HALO = 256
SK = SL + HALO          # 768 local keys (padded frame)
NQT = SL // 128         # 4 query tiles
NKT = SK // 128         # 6 key tiles
NET = E // 128          # 8 embed tiles
SCALE = 1.0 / math.sqrt(D)
NWARM = 12              # PE warm-up matmuls: a serial chain opens the
                        # HAM clock gate (~1.2GHz until it opens) early

# Bank-packed score layout for one head: each k-tile's whole contiguous
# query range is ONE matmul; the 6 ranges fill 3 PSUM banks of 512 f32
# exactly.  Each entry is one matmul: (bank, bank_col, kt, qlo, qhi).
SCORE_BLOCKS = [
    (0, 0,   2, 0,   384),
    (0, 384, 0, 0,   128),
    (1, 0,   3, 128, 512),
    (1, 384, 5, 384, 512),
    (2, 0,   1, 0,   256),
    (2, 256, 4, 256, 512),
]
# (kt, qtile) -> column offset of that 128-col block in the packed
# [128, 1536] probability tile.
PV_OFF = {}
for _b, _c, _kt, _qlo, _qhi in SCORE_BLOCKS:
    for _i in range((_qhi - _qlo) // 128):
        PV_OFF[(_kt, _qlo // 128 + _i)] = _b * 512 + _c + _i * 128


def _build_graph():
    nc = bacc.Bacc("TRN2", target_bir_lowering=False, debug=False, num_devices=NCORES)

    # ---- DRAM parameters (per-core shards staged by kernel()) ----
    # x packed as row tiles side by side: col kt*SK+s = x[kt*128+p, s]
    xall = nc.declare_dram_parameter("xall", [128, NET * SK], BF16, isOutput=False)
    # wq/wk packed as per-et panels: col et*E+kt*128+j = W[kt*128+p, et*128+j]
    wqp = nc.declare_dram_parameter("wqp", [128, NET * E], BF16, isOutput=False)
    wkp = nc.declare_dram_parameter("wkp", [128, NET * E], BF16, isOutput=False)
    # wv/wo packed as row tiles: col kt*E+c = W[kt*128+p, c]
    wvp = nc.declare_dram_parameter("wvp", [128, NET * E], BF16, isOutput=False)
    wop = nc.declare_dram_parameter("wop", [128, NET * E], BF16, isOutput=False)
    rt = nc.declare_dram_parameter("rt", [128, 128], BF16, isOutput=False)
    # bq in cols 0..7, bo' = Wo^T bv + bo in cols 8..15
    bqo = nc.declare_dram_parameter("bqo", [128, 2 * NET], F32, isOutput=False)
    csall = nc.declare_dram_parameter("csall", [128, 2 * SK], BF16, isOutput=False)
    maskpat = nc.declare_dram_parameter("maskpat", [128, 1536], BF16, isOutput=False)
    out_ext = nc.declare_dram_parameter("out", [E, SL], BF16, isOutput=True)

    with tile.TileContext(nc) as tc:
        with (
            tc.tile_pool(name="wpool", bufs=1) as wpool,
            tc.tile_pool(name="xpool", bufs=1) as xpool,
            tc.tile_pool(name="qk", bufs=3) as qkpool,
            tc.tile_pool(name="vpool", bufs=1) as vpool,
            tc.tile_pool(name="cs", bufs=1) as cspool,
            tc.tile_pool(name="small", bufs=1) as small,
            tc.tile_pool(name="rope", bufs=2) as ropepool,
            tc.tile_pool(name="pe", bufs=2) as pepool,
            tc.tile_pool(name="pm", bufs=16) as pmpool,
            tc.tile_pool(name="att", bufs=3) as attpool,
            tc.tile_pool(name="ctx", bufs=1) as ctxpool,
            tc.tile_pool(name="outp", bufs=8) as outpool,
            tc.tile_pool(name="mm", bufs=3, space="PSUM") as mmps,
            tc.tile_pool(name="sc", bufs=3, space="PSUM") as scps,
            tc.tile_pool(name="op", bufs=2, space="PSUM") as opps,
        ):
            # ---------- warm-up + DMA stream (consumption order) ----------
            warm_sb = small.tile([128, 512], BF16, tag="warm")
            nc.vector.memset(warm_sb[:], 0.0)
            ones_sb = small.tile([1, 64], BF16, tag="ones")
            nc.vector.memset(ones_sb[:], 1.0)

            warm_ctr = [0]

            def warm_mm():
                i = warm_ctr[0]
                warm_ctr[0] += 1
                wp = mmps.tile([128, 512], F32, tag="mm", name=f"warm{i}")
                nc.tensor.matmul(wp[:], warm_sb[:, 0:128], warm_sb[:],
                                 start=True, stop=True)

            bqo_sb = small.tile([128, 2 * NET], F32, tag="bqo")
            nc.sync.dma_start(bqo_sb[:], bqo[:])

            # x in 4 groups (kt 0, 1-2, 3-5, 6-7): first is small so the
            # first projection matmul can start as early as possible
            XG = [(0, 1), (1, 3), (3, 6), (6, 8)]
            xg_sb = []

            def x_mv(kt):
                for gi, (lo, hi) in enumerate(XG):
                    if lo <= kt < hi:
                        return xg_sb[gi], (kt - lo) * SK
                raise AssertionError

            def dma_xg(gi):
                lo, hi = XG[gi]
                t = xpool.tile([128, (hi - lo) * SK], BF16, tag=f"xg{gi}",
                               name=f"xg{gi}")
                nc.sync.dma_start(t[:], xall[:, lo * SK : hi * SK])
                xg_sb.append(t)

            wq_sb = [None] * NET
            wk_sb = [None] * NET

            def dma_panel(which, et):
                t = wpool.tile([128, E], BF16, tag=f"{'wq' if which == 'q' else 'wk'}{et}",
                               name=f"{which}p{et}")
                src = wqp if which == "q" else wkp
                nc.sync.dma_start(t[:], src[:, et * E : (et + 1) * E])
                (wq_sb if which == "q" else wk_sb)[et] = t

            # stream: bqo, xg0, wq0, xg1, wq1, xg2, wq2, wk0, cs, rt, wq3,
            # wk1, mask, wq4, wk2, wq5, wk3, wv0, wv1, wq6, wk4, wq7, wk5,
            # wk6, wk7, wo0..7 (wo reuses wq slots -> WAR-deferred)
            dma_xg(0)
            dma_panel("q", 0)
            dma_xg(1)
            dma_panel("q", 1)
            dma_xg(2)
            dma_panel("q", 2)
            dma_xg(3)
            dma_panel("k", 0)
            cs_sb = cspool.tile([128, 2 * SK], BF16, tag="cs")
            nc.sync.dma_start(cs_sb[:], csall[:])
            cos_sb = cs_sb[:, 0:SK]
            sin_sb = cs_sb[:, SK : 2 * SK]
            rt_sb = small.tile([128, 128], BF16, tag="rt")
            nc.sync.dma_start(rt_sb[:], rt[:])
            dma_panel("q", 3)
            dma_panel("k", 1)
            mask_sb = cspool.tile([128, 1536], BF16, tag="mask")
            nc.sync.dma_start(mask_sb[:], maskpat[:])
            dma_panel("q", 4)
            dma_panel("k", 2)
            dma_panel("q", 5)
            dma_panel("k", 3)
            wv_sb = []
            for g in range(2):
                t = vpool.tile([128, 4 * E], BF16, tag=f"wv{g}", name=f"wv{g}")
                nc.sync.dma_start(t[:], wvp[:, g * 4 * E : (g + 1) * 4 * E])
                wv_sb.append(t)
            dma_panel("q", 6)
            dma_panel("k", 4)
            dma_panel("q", 7)
            dma_panel("k", 5)
            dma_panel("k", 6)
            dma_panel("k", 7)
            # wo row tiles reuse the wq panel buffers (same tag): the DMA
            # waits until the last Q-projection matmul has read the panel.
            wo_sb = []
            for et in range(NET):
                t = wpool.tile([128, E], BF16, tag=f"wq{et}", name=f"wo{et}")
                nc.sync.dma_start(t[:], wop[:, et * E : (et + 1) * E])
                wo_sb.append(t)

            def wv_mv(kt, half):
                return wv_sb[kt // 4][:, (kt % 4) * E + half * 512 :
                                      (kt % 4) * E + (half + 1) * 512]

            # ---------- phase A: Q/K proj + rope + scores + exp + mask ----
            q_rope = [None] * NET   # rotating pool (bufs=3)
            k_rope = [None] * NET
            pm_tiles = {}

            def proj_q(et, trickle=()):
                qp = mmps.tile([128, 512], F32, tag="mm", name=f"qp{et}")
                for kt in range(NET):
                    xg, base = x_mv(kt)
                    nc.tensor.matmul(
                        qp[:],
                        wq_sb[et][:, kt * 128 : (kt + 1) * 128],
                        xg[:, base + HALO : base + SK],
                        start=(kt == 0),
                        stop=(kt == NET - 1),
                    )
                    if kt in trickle:
                        warm_mm()
                q_lin = ropepool.tile([128, SL], BF16, tag="qlin", bufs=3)
                nc.scalar.activation(
                    q_lin[:], qp[:],
                    mybir.ActivationFunctionType.Identity,
                    bias=bqo_sb[:, et : et + 1], scale=1.0,
                )
                return q_lin

            q_lin_t = [None] * NET

            def proj_k_rope(et):
                # k: all SK rows, no bias
                kp = mmps.tile([128, 512], F32, tag="mm", name=f"kp{et}")
                kp2 = mmps.tile([128, 512], F32, tag="mm", name=f"kp2{et}")
                for kt in range(NET):
                    xg, base = x_mv(kt)
                    nc.tensor.matmul(
                        kp[:],
                        wk_sb[et][:, kt * 128 : (kt + 1) * 128],
                        xg[:, base : base + 512],
                        start=(kt == 0), stop=(kt == NET - 1),
                    )
                for kt in range(NET):
                    xg, base = x_mv(kt)
                    nc.tensor.matmul(
                        kp2[:, 0 : SK - 512],
                        wk_sb[et][:, kt * 128 : (kt + 1) * 128],
                        xg[:, base + 512 : base + SK],
                        start=(kt == 0), stop=(kt == NET - 1),
                    )
                k_lin = ropepool.tile([128, SK], BF16, tag="klin")
                nc.scalar.copy(k_lin[:, 0:512], kp[:, 0:512])
                nc.scalar.copy(k_lin[:, 512:SK], kp2[:, 0 : SK - 512])
                q_lin = q_lin_t[et]
                # rotate_half via matmul with the static rotation matrix
                rotp = mmps.tile([128, 512], F32, tag="mm", name="rotp")
                nc.tensor.matmul(rotp[:, 0:SL], rt_sb[:], q_lin[:],
                                 start=True, stop=True)
                rotk = mmps.tile([128, 512], F32, tag="mm", name="rotk")
                nc.tensor.matmul(rotk[:, :], rt_sb[:], k_lin[:, 0:512],
                                 start=True, stop=True)
                rotk2 = mmps.tile([128, 512], F32, tag="mm", name="rotk2")
                nc.tensor.matmul(rotk2[:, 0 : SK - 512], rt_sb[:],
                                 k_lin[:, 512:SK], start=True, stop=True)
                # all-bf16 SBUF temporaries hit the DVE 2x/4x perf modes
                t1 = ropepool.tile([128, SL], BF16, tag="t1", name="t1")
                nc.vector.tensor_mul(t1[:], q_lin[:], cos_sb[:, HALO:SK])
                t2 = ropepool.tile([128, SL], BF16, tag="t2", name="t2")
                nc.vector.tensor_mul(t2[:], rotp[:, 0:SL], sin_sb[:, HALO:SK])
                qf = qkpool.tile([128, SL], BF16, tag="qr", name=f"qf{et}")
                nc.vector.tensor_add(qf[:], t1[:], t2[:])
                q_rope[et] = qf
                t3 = ropepool.tile([128, SK], BF16, tag="t3", name="t3")
                nc.vector.tensor_mul(t3[:], k_lin[:], cos_sb[:])
                t4 = ropepool.tile([128, SK], BF16, tag="t4", name="t4")
                nc.vector.tensor_mul(t4[:, 0:512], rotk[:, :], sin_sb[:, 0:512])
                nc.vector.tensor_mul(t4[:, 512:SK], rotk2[:, 0 : SK - 512],
                                     sin_sb[:, 512:SK])
                kf = qkpool.tile([128, SK], BF16, tag="kr", name=f"kf{et}")
                nc.vector.tensor_add(kf[:], t3[:], t4[:])
                k_rope[et] = kf

            def emit_scores(hp, sub):
                # one head: 8 matmuls bank-pack the 12 band blocks into 3
                # PSUM banks; 3 big exps; one batched mask multiply.
                et = hp
                banks = [scps.tile([128, 512], F32, tag="sc",
                                   name=f"sc{hp}_{sub}_{b}") for b in range(3)]
                for (b, c, kt, qlo, qhi) in SCORE_BLOCKS:
                    nc.tensor.matmul(
                        banks[b][:, c : c + (qhi - qlo)],
                        k_rope[et][sub : sub + 64, kt * 128 : (kt + 1) * 128],
                        q_rope[et][sub : sub + 64, qlo:qhi],
                        start=True, stop=True,
                    )
                pe = pepool.tile([128, 1536], BF16, tag="pe",
                                 name=f"pe{hp}_{sub}")
                for b in range(3):
                    nc.scalar.activation(
                        pe[:, b * 512 : (b + 1) * 512], banks[b][:],
                        mybir.ActivationFunctionType.Exp,
                        bias=0.0, scale=SCALE,
                    )
                pm = pmpool.tile([128, 1536], BF16, tag="pm",
                                 name=f"pm{hp}_{sub}")
                nc.vector.tensor_mul(pm[:], pe[:], mask_sb[:])
                pm_tiles[(hp, sub)] = pm

            for _ in range(NWARM):
                warm_mm()
            q_lin_t[0] = proj_q(0, trickle=(1, 3, 5))
            q_lin_t[1] = proj_q(1, trickle=(2, 5))
            for et in range(NET):
                if et + 2 < NET:
                    q_lin_t[et + 2] = proj_q(et + 2,
                                             trickle=(3,) if et <= 1 else ())
                proj_k_rope(et)
                if et >= 1:
                    emit_scores(et - 1, 0)
                    emit_scores(et - 1, 64)
            emit_scores(NET - 1, 0)
            emit_scores(NET - 1, 64)

            # ---------- phase B: V proj + PV + out projection ----------
            # Per head 128 cols: ones at col 0 (so PV's sum(P) row lands at
            # PSUM partition 0, where the custom reciprocal can read it
            # directly) and the 64 value dims at cols 64..127 (so the
            # context evacuates via an ACT partition-shift copy 64->0).
            # Cols 1..63 stay garbage -- their PSUM rows are never read.
            v_sb = []
            for st in range(NKT):
                vt = vpool.tile([128, 16 * 128], BF16, tag=f"v{st}",
                                name=f"v{st}")
                nc.gpsimd.memset(
                    vt[:].rearrange("p (h c) -> p h c", c=128)[:, :, 0:1], 1.0
                )
                v_sb.append(vt)

            def emit_v(st):
                vt = v_sb[st]
                for half in range(2):
                    vp = mmps.tile([128, 512], F32, tag="mm", name=f"vp{st}")
                    for kt in range(NET):
                        xg, base = x_mv(kt)
                        nc.tensor.matmul(
                            vp[:],
                            xg[:, base + st * 128 : base + (st + 1) * 128],
                            wv_mv(kt, half),
                            start=(kt == 0),
                            stop=(kt == NET - 1),
                        )
                    dst = vt[:, half * 8 * 128 : (half * 8 + 8) * 128].rearrange(
                        "p (h c) -> p h c", c=128
                    )[:, :, 64:128]
                    nc.scalar.copy(dst, vp[:].rearrange("p (h c) -> p h c", c=64))

            ctx_sb = [ctxpool.tile([128, SL], BF16, tag=f"ctx{et}",
                                   name=f"ctx{et}")
                      for et in range(NET)]

            pend_lbc = []

            def pv_alloc(hp, sub):
                return scps.tile([128, 512], F32, tag="sc",
                                 name=f"cx{hp}_{sub}")

            def pv_mm_range(cxp, hp, sub, kts):
                pm = pm_tiles[(hp, sub)]
                h = 2 * hp + sub // 64
                for kt in kts:
                    lo = max(0, kt - 2) * 128
                    hi = min(kt + 1, NQT) * 128
                    off = PV_OFF[(kt, lo // 128)]
                    nc.tensor.matmul(
                        cxp[:, lo:hi],
                        v_sb[kt][:, h * 128 : (h + 1) * 128],
                        pm[:, off : off + (hi - lo)],
                        start=(kt == 0), stop=(kt == NKT - 1),
                    )

            def pv_finish(cxp, hp, sub):
                pm_tiles.pop((hp, sub))
                # sum(P) sits at PSUM partition 0: the custom reciprocal
                # reads it in place (no single-partition DVE copy)
                linv = attpool.tile([1, SL], F32, tag="linv")
                nc.vector.reciprocal_approx_fast(linv[:], cxp[0:1, :])
                if (2 * hp + sub // 64) % 2 == 0:
                    # even units: GpSimd broadcast issued immediately --
                    # its 1.5us runs off every critical queue and needs
                    # no PE matmul and no bf16 cast
                    lbc = attpool.tile([64, SL], F32, tag="lbc", bufs=3)
                    nc.gpsimd.partition_broadcast(lbc[:], linv[:])
                else:
                    # odd units: PE broadcast at flush time; cast on ACT
                    lbc = attpool.tile([1, SL], BF16, tag="linvb")
                    nc.scalar.copy(lbc[:], linv[:])
                # context rows evacuate 64->0 via ACT partition-shift copy
                craw = attpool.tile([64, SL], F32, tag="craw")
                nc.scalar.copy(craw[:], cxp[64:128, :])
                pend_lbc.append((hp, sub, lbc, craw))

            def flush_lbc():
                """Finish ctx = craw * (1/l).  Only called with >=1 unit of
                PE work emitted since the PV (covers the recip chain)."""
                hp, sub, lbc, craw = pend_lbc.pop(0)
                if (2 * hp + sub // 64) % 2 == 0:
                    nc.vector.tensor_mul(ctx_sb[hp][sub : sub + 64, :],
                                         craw[:], lbc[:])
                else:
                    lbc_ps = scps.tile([128, 512], F32, tag="sc",
                                       name=f"lbc{hp}_{sub}")
                    nc.tensor.matmul(lbc_ps[0:64, :], ones_sb[:],
                                     lbc[:], start=True, stop=True)
                    nc.vector.tensor_mul(ctx_sb[hp][sub : sub + 64, :],
                                         craw[:], lbc_ps[0:64, :])

            def emit_pv_mm(hp, sub):
                """PV matmuls + start of the normalize chain (recip)."""
                cxp = pv_alloc(hp, sub)
                pv_mm_range(cxp, hp, sub, range(NKT))
                pv_finish(cxp, hp, sub)

            # V projections with the first PV pair's matmuls threaded in:
            # each PV matmul reads v_sb[kt] evacuated two V units earlier,
            # so the junction has no exposed evac wait.
            emit_v(0)
            emit_v(1)
            cx00 = pv_alloc(0, 0)
            emit_v(2)
            pv_mm_range(cx00, 0, 0, [0])
            emit_v(3)
            pv_mm_range(cx00, 0, 0, [1])
            cx064 = pv_alloc(0, 64)
            pv_mm_range(cx064, 0, 64, [0])
            emit_v(4)
            pv_mm_range(cx00, 0, 0, [2])
            pv_mm_range(cx064, 0, 64, [1])
            emit_v(5)
            pv_mm_range(cx00, 0, 0, [3, 4, 5])
            pv_finish(cx00, 0, 0)
            pv_mm_range(cx064, 0, 64, [2, 3, 4, 5])
            pv_finish(cx064, 0, 64)

            # Out-projection partials, spread through the PV phase as lbc
            # cover: 5 held PSUM banks accumulate et chunks as ctx tiles
            # complete (et 0-2 after ctx[2], et 3-5 after ctx[5]).
            op_hold = {}

            def op_chunk(eo, ets, pool=None):
                op = op_hold.get(eo)
                if op is None:
                    op = pool.tile([128, 512], F32,
                                   tag=("op" if pool is opps else "mm"),
                                   name=f"op{eo}")
                    op_hold[eo] = op
                for et in ets:
                    nc.tensor.matmul(
                        op[:],
                        wo_sb[et][:, eo * 128 : (eo + 1) * 128],
                        ctx_sb[et][:],
                        start=(et == 0), stop=False,
                    )

            # Pair k's broadcast flushes after pair k+1's first PV unit
            # (~1.5 units of cover for the recip->cast chain); out partial
            # chunks interleave as additional cover once ctx tiles land.
            OPPOOL = {0: opps, 1: opps, 2: mmps, 3: mmps, 4: mmps}

            def opA(eo):
                op_chunk(eo, range(0, 3), pool=OPPOOL[eo])

            def opB(eo):
                op_chunk(eo, range(3, 6), pool=OPPOOL[eo])

            # lbc lags two units: the recip->cast->broadcast chain gets
            # ~2us of PV/partial matmul cover before the PE needs it
            cover = iter(
                [None] * 6   # until ctx[2] is complete (flush of (2,64))
                + [("A", 0), ("A", 1), ("A", 2), ("A", 3), ("A", 4), None,
                   ("B", 0), ("B", 1), ("B", 2), ("B", 3), ("B", 4)]
            )

            def emit_cover():
                c = next(cover, None)
                if c is None:
                    return
                stage, eo = c
                (opA if stage == "A" else opB)(eo)

            for hp in range(1, NET):
                for sub in (0, 64):
                    emit_pv_mm(hp, sub)
                    if len(pend_lbc) > 2:
                        emit_cover()
                        flush_lbc()
            emit_cover()
            flush_lbc()
            emit_cover()
            flush_lbc()
            for _ in range(10):
                emit_cover()

            def finish_out(eo, op):
                o_sb = outpool.tile([128, SL], BF16, tag="o")
                if eo % 2 == 0:
                    nc.scalar.activation(
                        o_sb[:], op[:], mybir.ActivationFunctionType.Identity,
                        bias=bqo_sb[:, NET + eo : NET + eo + 1], scale=1.0,
                    )
                else:
                    nc.vector.tensor_scalar_add(
                        o_sb[:], op[:], bqo_sb[:, NET + eo : NET + eo + 1],
                    )
                nc.sync.dma_start(out_ext[eo * 128 : (eo + 1) * 128, :], o_sb[:])

            def tail_warm():
                wp = scps.tile([128, 512], F32, tag="sc",
                               name=f"twarm{warm_ctr[0]}")
                warm_ctr[0] += 1
                nc.tensor.matmul(wp[:], warm_sb[:, 0:128], warm_sb[:],
                                 start=True, stop=True)

            # rank updates et=6,7 for the held banks, then evacuate; tail
            # warm matmuls keep the HAM clock open while ACT/DVE/DMA drain
            for eo in range(5):
                op = op_hold[eo]
                for et in (6, 7):
                    nc.tensor.matmul(
                        op[:],
                        wo_sb[et][:, eo * 128 : (eo + 1) * 128],
                        ctx_sb[et][:],
                        start=False, stop=(et == 7),
                    )
                finish_out(eo, op)
                tail_warm()
            # eo=5..7: full chains through freed banks
            for eo, pool, tg in ((5, mmps, "mm"), (6, opps, "op"), (7, opps, "op")):
                op = pool.tile([128, 512], F32, tag=tg, name=f"opf{eo}")
                for et in range(NET):
                    nc.tensor.matmul(
                        op[:],
                        wo_sb[et][:, eo * 128 : (eo + 1) * 128],
                        ctx_sb[et][:],
                        start=(et == 0), stop=(et == NET - 1),
                    )
                finish_out(eo, op)
                tail_warm()
            tail_warm()
            tail_warm()

    nc.compile()
    return nc


_NC_CACHE = None
LAST_RESULT = None


def _get_graph():
    global _NC_CACHE
    if _NC_CACHE is None:
        _NC_CACHE = _build_graph()
    return _NC_CACHE


def _rot_matrix():
    # rot(q)[d] = -q[d+32] (d<32) ; q[d-32] (d>=32), per 64-block; 2 blocks.
    r64 = np.zeros((64, 64), dtype=np.float32)
    for d in range(32):
        r64[d, d + 32] = -1.0
        r64[d + 32, d] = 1.0
    r = np.zeros((128, 128), dtype=np.float32)
    r[0:64, 0:64] = r64
    r[64:128, 64:128] = r64
    return r


def _maskpat(core):
    """Packed [128, 1536] multiplicative window mask for one core.

    Column b*512 + c + i*128 + u corresponds to key row ki of k-tile kt
    against query column (qlo//128 + i)*128 + u; valid iff the key is in
    the causal 256-window and (core 0) not a zero-padded halo row.
    """
    pat = np.zeros((128, 1536), dtype=np.float32)
    ki = np.arange(128)[:, None]
    u = np.arange(128)[None, :]
    for (b, c, kt, qlo, qhi) in SCORE_BLOCKS:
        for i in range((qhi - qlo) // 128):
            qj = qlo + i * 128 + u
            k_pad = kt * 128 + ki
            valid = (qj <= k_pad) & (k_pad <= qj + HALO)
            if core == 0:
                valid = valid & (k_pad >= HALO)
            pat[:, b * 512 + c + i * 128 : b * 512 + c + (i + 1) * 128] = valid
    return pat.astype(ml_dtypes.bfloat16)


def kernel(x, mask, cos, sin, Wq, bq, Wk, Wv, bv, Wo, bo):
    x = np.asarray(x, dtype=np.float32)
    cos = np.asarray(cos, dtype=np.float32)
    sin = np.asarray(sin, dtype=np.float32)
    B = x.shape[0]
    assert (B, S, E) == x.shape

    bf = lambda a: np.ascontiguousarray(a).astype(ml_dtypes.bfloat16)
    Wq = np.asarray(Wq, np.float32)
    Wk = np.asarray(Wk, np.float32)
    Wv = np.asarray(Wv, np.float32)
    Wo = np.asarray(Wo, np.float32)
    # per-et panels: [p, et, kt, j]
    wqp_b = bf(Wq.reshape(NET, 128, NET, 128).transpose(1, 2, 0, 3)
               .reshape(128, NET * E))
    wkp_b = bf(Wk.reshape(NET, 128, NET, 128).transpose(1, 2, 0, 3)
               .reshape(128, NET * E))
    # row-tile packs: [p, kt, c]
    wvp_b = bf(Wv.reshape(NET, 128, E).transpose(1, 0, 2).reshape(128, NET * E))
    wop_b = bf(Wo.reshape(NET, 128, E).transpose(1, 0, 2).reshape(128, NET * E))
    rt_b = bf(_rot_matrix().T)
    # fold the V bias through the output projection: ctx rows sum to 1
    bo_f = np.asarray(bo, np.float32) + Wo.T @ np.asarray(bv, np.float32)
    bqo_t = np.concatenate(
        [np.asarray(bq, np.float32).reshape(NET, 128).T,
         bo_f.reshape(NET, 128).T], axis=1)
    bqo_t = np.ascontiguousarray(bqo_t)

    in_maps = []
    for c in range(NCORES):
        lo = c * SL - HALO
        xp = np.zeros((SK, E), dtype=np.float32)
        cp = np.zeros((SK, D), dtype=np.float32)
        sp = np.zeros((SK, D), dtype=np.float32)
        src_lo = max(lo, 0)
        dst_lo = src_lo - lo
        xp[dst_lo:] = x[0, src_lo : lo + SK]
        cp[dst_lo:] = cos[0, src_lo : lo + SK]
        sp[dst_lo:] = sin[0, src_lo : lo + SK]
        xall_b = bf(xp.T.reshape(NET, 128, SK).transpose(1, 0, 2)
                    .reshape(128, NET * SK))
        cs_b = np.concatenate(
            [np.tile(cp.T, (2, 1)), np.tile(sp.T, (2, 1))], axis=1)
        in_maps.append({
            "xall": xall_b,
            "wqp": wqp_b, "wkp": wkp_b, "wvp": wvp_b, "wop": wop_b,
            "rt": rt_b,
            "bqo": bqo_t,
            "csall": bf(cs_b),
            "maskpat": _maskpat(c),
        })

    nc = _get_graph()
    trace = bool(os.environ.get("BASS_KERNEL_TRACE"))
    if trace:
        _ensure_ntff_hook()
    res = run_bass_kernel_spmd(
        nc, in_maps, core_ids=list(range(NCORES)), trace=trace
    )
    global LAST_RESULT
    LAST_RESULT = res

    out = np.empty((1, S, E), dtype=np.float32)
    for c in range(NCORES):
        out[0, c * SL : (c + 1) * SL, :] = (
            res.results[c]["out"].astype(np.float32).T)
    return out


if __name__ == "__main__":
    import reference
    inputs = reference.setup_inputs()
    inputs = {k: np.asarray(v) for k, v in inputs.items()}
    got = kernel(**inputs)
    exp = np.asarray(reference.reference(**inputs))
    err = np.abs(got - exp).max() / np.abs(exp).max()
    print("rel err:", err)
